# revision 3
# baseline (speedup 1.0000x reference)
"""GCN layer kernel for 8 Trainium2 NeuronCores.

Computes: out = relu(dinv[:,None] * ((adj+I).T @ (dinv[:,None] * (x@W))) + b)
where dinv = rsqrt(colsum(adj) + 1).

Strategy: shard adj by COLUMNS across the 8 cores. Column block c (together
with the full scaled source features z) is exactly what's needed to produce
output rows [c*2048, (c+1)*2048) -- so no device collectives are needed.
Host preprocessing folds the self-loop (+I) and the output-side dinv scaling
into the adjacency block, and casts it to bf16 (exactly halves the HBM
traffic, which is the roofline for this problem: 16384^2 matrix vs 64-wide
features). Each core then runs a single streaming matmul pass:

    out_c.T[64, 2048] = sum_k z_k.T[64,128] @ As_k[128, 2048]   (f32 PSUM)
    out_c.T = relu(out_c.T + b)                                  (one ACT op)

z (16384x64) is the stationary operand (ldweights), the 1 GB adjacency
streams through as the moving operand straight from contiguous DMA tiles.
"""

import sys

import numpy as np

if "/opt/trn_rl_repo" not in sys.path:
    sys.path.insert(0, "/opt/trn_rl_repo")

import ml_dtypes

N = 16384
F = 64
NCORES = 8
NB = N // NCORES  # 2048 columns (= output rows) per core
P = 128
KT = N // P  # 128 k-tiles of 128 source rows each
MM_N = 512  # moving-operand free dim per matmul (one PSUM bank of f32)

_BASS_CACHE: dict = {}


def _build_bass():
    if "nc" in _BASS_CACHE:
        return _BASS_CACHE["nc"]

    import concourse.mybir as mybir
    import concourse.tile as tile
    from concourse import bacc

    nc = bacc.Bacc("TRN2", target_bir_lowering=False, debug=False,
                   num_devices=NCORES)

    a_in = nc.dram_tensor("a", [N, NB], mybir.dt.bfloat16, kind="ExternalInput")
    z_in = nc.dram_tensor("z", [P, KT * F], mybir.dt.bfloat16,
                          kind="ExternalInput")
    b_in = nc.dram_tensor("bvec", [F, 1], mybir.dt.float32,
                          kind="ExternalInput")
    o_out = nc.dram_tensor("o", [F, NB], mybir.dt.float32,
                           kind="ExternalOutput")

    a_tiles = a_in.ap().rearrange("(kt p) i -> kt p i", p=P)  # [KT, 128, NB]

    with tile.TileContext(nc) as tc:
        with (
            tc.tile_pool(name="singles", bufs=1) as singles,
            tc.tile_pool(name="apool", bufs=6) as apool,
            tc.tile_pool(name="psum", bufs=1, space="PSUM") as psum_pool,
        ):
            z_sb = singles.tile([P, KT * F], mybir.dt.bfloat16)
            nc.sync.dma_start(z_sb[:], z_in.ap())
            b_sb = singles.tile([F, 1], mybir.dt.float32)
            nc.sync.dma_start(b_sb[:], b_in.ap())

            ps = psum_pool.tile([F, NB], mybir.dt.float32)  # 4 PSUM banks

            for kt in range(KT):
                at = apool.tile([P, NB], mybir.dt.bfloat16)
                nc.sync.dma_start(at[:], a_tiles[kt])
                for nn in range(NB // MM_N):
                    nc.tensor.matmul(
                        ps[:, nn * MM_N:(nn + 1) * MM_N],
                        lhsT=z_sb[:, kt * F:(kt + 1) * F],
                        rhs=at[:, nn * MM_N:(nn + 1) * MM_N],
                        start=(kt == 0),
                        stop=(kt == KT - 1),
                    )

            out_sb = singles.tile([F, NB], mybir.dt.float32)
            nc.scalar.activation(out_sb[:], ps[:],
                                 mybir.ActivationFunctionType.Relu,
                                 bias=b_sb[:], scale=1.0)
            nc.sync.dma_start(o_out.ap(), out_sb[:])

    nc.compile()
    _BASS_CACHE["nc"] = nc
    return nc


def _host_prep(x, adj, W, b):
    """Host-side sharding/preprocessing -> per-core input maps."""
    x = np.asarray(x, dtype=np.float32)
    adj = np.asarray(adj, dtype=np.float32)
    W = np.asarray(W, dtype=np.float32)
    b = np.asarray(b, dtype=np.float32)

    deg = adj.sum(axis=0) + 1.0
    dinv = np.where(deg > 0, 1.0 / np.sqrt(deg), 0.0).astype(np.float32)

    z = (dinv[:, None] * (x @ W)).astype(np.float32)  # [N, F]
    # k-major layout: z_sb[p, kt*F + f] = z[kt*128 + p, f]
    z_dev = np.ascontiguousarray(
        z.reshape(KT, P, F).transpose(1, 0, 2).reshape(P, KT * F)
    ).astype(ml_dtypes.bfloat16)

    b_dev = np.ascontiguousarray(b.reshape(F, 1))

    in_maps = []
    idx = np.arange(NB)
    for c in range(NCORES):
        cs = c * NB
        blk = adj[:, cs:cs + NB] * dinv[cs:cs + NB][None, :]
        blk[cs + idx, idx] += dinv[cs + idx]  # fold self-loop (+I)
        in_maps.append({
            "a": blk.astype(ml_dtypes.bfloat16),
            "z": z_dev,
            "bvec": b_dev,
        })
    return in_maps


def kernel(x, adj, W, b):
    from concourse import bass_utils

    nc = _build_bass()
    in_maps = _host_prep(x, adj, W, b)
    res = bass_utils.run_bass_kernel_spmd(nc, in_maps,
                                          core_ids=list(range(NCORES)))
    out = np.empty((N, F), dtype=np.float32)
    for c in range(NCORES):
        out[c * NB:(c + 1) * NB, :] = res.results[c]["o"].T
    return out


# revision 5
# speedup vs baseline: 8.2921x; 8.2921x over previous
"""GCN layer kernel for 8 Trainium2 NeuronCores.

Computes: out = relu(dinv[:,None] * ((adj+I).T @ (dinv[:,None] * (x@W))) + b)
where dinv = rsqrt(colsum(adj) + 1).

Strategy: shard adj by COLUMNS across the 8 cores. Column block c (together
with the full scaled source features z) is exactly what's needed to produce
output rows [c*2048, (c+1)*2048) -- so no device collectives are needed.
Host preprocessing folds the self-loop (+I) and the output-side dinv scaling
into the adjacency block, and casts it to bf16 (exactly halves the HBM
traffic, which is the roofline for this problem: 16384^2 matrix vs 64-wide
features). Each core then runs a single streaming matmul pass:

    out_c.T[64, 2048] = sum_k z_k.T[64,128] @ As_k[128, 2048]   (f32 PSUM)
    out_c.T = relu(out_c.T + b)                                  (one ACT op)

z (16384x64) is the stationary operand (ldweights), the 1 GB adjacency
streams through as the moving operand straight from contiguous DMA tiles.
"""

import sys

import numpy as np

if "/opt/trn_rl_repo" not in sys.path:
    sys.path.insert(0, "/opt/trn_rl_repo")

import ml_dtypes

N = 16384
F = 64
NCORES = 8
NB = N // NCORES  # 2048 columns (= output rows) per core
P = 128
KT = N // P  # 128 k-tiles of 128 source rows each
MM_N = 512  # moving-operand free dim per matmul (one PSUM bank of f32)

_BASS_CACHE: dict = {}


def _build_bass(reps: int = 1):
    """Build the per-core Bass module. reps>1 repeats the whole compute
    (same inputs/outputs) inside one NEFF -- used only for benchmarking
    device time independent of dispatch overhead."""
    if reps in _BASS_CACHE:
        return _BASS_CACHE[reps]

    import concourse.mybir as mybir
    import concourse.tile as tile
    from concourse import bacc

    nc = bacc.Bacc("TRN2", target_bir_lowering=False, debug=False,
                   num_devices=NCORES)

    a_in = nc.dram_tensor("a", [N, NB], mybir.dt.bfloat16, kind="ExternalInput")
    z_in = nc.dram_tensor("z", [P, KT * F], mybir.dt.bfloat16,
                          kind="ExternalInput")
    b_in = nc.dram_tensor("bvec", [F, 1], mybir.dt.float32,
                          kind="ExternalInput")
    o_out = nc.dram_tensor("o", [F, NB], mybir.dt.float32,
                           kind="ExternalOutput")

    a_tiles = a_in.ap().rearrange("(kt p) i -> kt p i", p=P)  # [KT, 128, NB]

    with tile.TileContext(nc) as tc:
        with (
            tc.tile_pool(name="singles", bufs=1) as singles,
            tc.tile_pool(name="apool", bufs=6) as apool,
            tc.tile_pool(name="psum", bufs=1, space="PSUM") as psum_pool,
        ):
            z_sb = singles.tile([P, KT * F], mybir.dt.bfloat16)
            nc.sync.dma_start(z_sb[:], z_in.ap())
            b_sb = singles.tile([F, 1], mybir.dt.float32)
            nc.sync.dma_start(b_sb[:], b_in.ap())

            for _rep in range(reps):
                ps = psum_pool.tile([F, NB], mybir.dt.float32)  # 4 PSUM banks

                for kt in range(KT):
                    at = apool.tile([P, NB], mybir.dt.bfloat16)
                    nc.sync.dma_start(at[:], a_tiles[kt])
                    for nn in range(NB // MM_N):
                        nc.tensor.matmul(
                            ps[:, nn * MM_N:(nn + 1) * MM_N],
                            lhsT=z_sb[:, kt * F:(kt + 1) * F],
                            rhs=at[:, nn * MM_N:(nn + 1) * MM_N],
                            start=(kt == 0),
                            stop=(kt == KT - 1),
                        )

                out_sb = singles.tile([F, NB], mybir.dt.float32,
                                      tag="out_sb")
                nc.scalar.activation(out_sb[:], ps[:],
                                     mybir.ActivationFunctionType.Relu,
                                     bias=b_sb[:], scale=1.0)
                nc.sync.dma_start(o_out.ap(), out_sb[:])

    nc.compile()
    _BASS_CACHE[reps] = nc
    return nc


def _host_prep(x, adj, W, b):
    """Host-side sharding/preprocessing -> per-core input maps."""
    x = np.asarray(x, dtype=np.float32)
    adj = np.asarray(adj, dtype=np.float32)
    W = np.asarray(W, dtype=np.float32)
    b = np.asarray(b, dtype=np.float32)

    deg = adj.sum(axis=0) + 1.0
    dinv = np.where(deg > 0, 1.0 / np.sqrt(deg), 0.0).astype(np.float32)

    z = (dinv[:, None] * (x @ W)).astype(np.float32)  # [N, F]
    # k-major layout: z_sb[p, kt*F + f] = z[kt*128 + p, f]
    z_dev = np.ascontiguousarray(
        z.reshape(KT, P, F).transpose(1, 0, 2).reshape(P, KT * F)
    ).astype(ml_dtypes.bfloat16)

    b_dev = np.ascontiguousarray(b.reshape(F, 1))

    in_maps = []
    idx = np.arange(NB)
    for c in range(NCORES):
        cs = c * NB
        blk = adj[:, cs:cs + NB] * dinv[cs:cs + NB][None, :]
        blk[cs + idx, idx] += dinv[cs + idx]  # fold self-loop (+I)
        in_maps.append({
            "a": blk.astype(ml_dtypes.bfloat16),
            "z": z_dev,
            "bvec": b_dev,
        })
    return in_maps


def kernel(x, adj, W, b):
    from concourse import bass_utils

    nc = _build_bass()
    in_maps = _host_prep(x, adj, W, b)
    res = bass_utils.run_bass_kernel_spmd(nc, in_maps,
                                          core_ids=list(range(NCORES)))
    out = np.empty((N, F), dtype=np.float32)
    for c in range(NCORES):
        out[c * NB:(c + 1) * NB, :] = res.results[c]["o"].T
    return out


# revision 37
# speedup vs baseline: 22.5522x; 2.7197x over previous
"""GCN layer kernel for 8 Trainium2 NeuronCores.

Computes: out = relu(dinv[:,None] * ((adj+I).T @ (dinv[:,None] * (x@W))) + b)
where dinv = rsqrt(colsum(adj) + 1).

Strategy: shard adj by COLUMNS across the 8 cores. Column block c (together
with the full scaled source features z) is exactly what's needed to produce
output rows [c*2048, (c+1)*2048) -- so no device collectives are needed.
Host preprocessing folds the self-loop (+I) and the output-side dinv scaling
into the adjacency block, and casts it to bf16 (exactly halves the HBM
traffic, which is the roofline for this problem: 16384^2 matrix vs 64-wide
features). Each core then runs a single streaming matmul pass:

    out_c.T[64, 2048] = sum_k z_k.T[64,128] @ As_k[128, 2048]   (f32 PSUM)
    out_c.T = relu(out_c.T + b)                                  (one ACT op)

z (16384x64) is the stationary operand (ldweights), the 1 GB adjacency
streams through as the moving operand straight from contiguous DMA tiles.
"""

import sys

import numpy as np

if "/opt/trn_rl_repo" not in sys.path:
    sys.path.insert(0, "/opt/trn_rl_repo")

import ml_dtypes

N = 16384
F = 64
NCORES = 8
NB = N // NCORES  # 2048 columns (= output rows) per core
P = 128
KT = N // P  # 128 k-tiles of 128 source rows each
MM_N = 512  # moving-operand free dim per matmul (one PSUM bank of f32)
DMA_BATCH = 2  # k-tiles per dma_start
APOOL_BUFS = 6  # in-flight A-tile slots (prefetch depth)
MODE = "fp8pair"  # variant kernel() uses

_BASS_CACHE: dict = {}


def _build_bass(reps: int = 1, mode: str = "full"):
    """Build the per-core Bass module. reps>1 repeats the whole compute
    (same inputs/outputs) inside one NEFF -- used only for benchmarking
    device time independent of dispatch overhead. mode: "full" | "dma"
    (loads only, no matmul) | "mm" (matmuls from a single resident tile,
    1/128th of the DMA traffic)."""
    key = (reps, mode, DMA_BATCH, APOOL_BUFS)
    if key in _BASS_CACHE:
        return _BASS_CACHE[key]

    import concourse.mybir as mybir
    import concourse.tile as tile
    from concourse import bacc

    nc = bacc.Bacc("TRN2", target_bir_lowering=False, debug=False,
                   num_devices=NCORES)

    fp8 = mode in ("fp8", "fp8pair", "dma8", "mm8")
    pair = mode in ("pair", "fp8pair")
    a_dt = mybir.dt.float8e4 if fp8 else mybir.dt.bfloat16
    # pair mode: two col-group-tiled matmuls run concurrently. Each 512-chunk
    # nn gets its own PSUM bank (columns nn*512) with even chunks on
    # partitions 0-63 and odd chunks on 64-127, so no two accumulation
    # groups share a bank.
    b_p = 2 * F if pair else F       # bias/dinv partition count
    a_in = nc.dram_tensor("a", [N, NB], a_dt, kind="ExternalInput")
    z_in = nc.dram_tensor("z", [P, KT * F], mybir.dt.bfloat16,
                          kind="ExternalInput")
    b_in = nc.dram_tensor("bvec", [b_p, 1], mybir.dt.float32,
                          kind="ExternalInput")
    if fp8:
        d_in = nc.dram_tensor("dinv", [b_p, NB], mybir.dt.float32,
                              kind="ExternalInput")
    o_out = nc.dram_tensor("o", [F, NB], mybir.dt.float32,
                           kind="ExternalOutput")

    kb = DMA_BATCH
    # [KT/kb, 128, kb, NB]: group kb consecutive k-tiles into one DMA
    a_tiles = a_in.ap().rearrange("(g t p) i -> g p t i", t=kb, p=P)

    with tile.TileContext(nc) as tc:
        with (
            tc.tile_pool(name="singles", bufs=1) as singles,
            tc.tile_pool(name="apool", bufs=APOOL_BUFS) as apool,
            tc.tile_pool(name="psum", bufs=1, space="PSUM") as psum_pool,
        ):
            z_sb = singles.tile([P, KT * F], mybir.dt.bfloat16)
            nc.sync.dma_start(z_sb[:], z_in.ap())
            b_sb = singles.tile([b_p, 1], mybir.dt.float32)
            nc.sync.dma_start(b_sb[:], b_in.ap())
            d_sb = None
            if fp8:
                d_sb = singles.tile([b_p, NB], mybir.dt.float32, tag="d_sb")
                nc.sync.dma_start(d_sb[:], d_in.ap())

            mm_tile = None
            if mode in ("mm", "mm8"):
                mm_tile = singles.tile([P, kb, NB], a_dt, tag="mm_tile")
                nc.sync.dma_start(mm_tile[:], a_tiles[0])

            for _rep in range(reps):
                ps = psum_pool.tile([b_p, NB], mybir.dt.float32)

                for g in range(KT // kb):
                    if mode in ("mm", "mm8"):
                        at = mm_tile
                    else:
                        at = apool.tile([P, kb, NB], a_dt)
                        nc.sync.dma_start(at[:], a_tiles[g])
                    if mode in ("dma", "dma8"):
                        continue
                    for t in range(kb):
                        kt = g * kb + t
                        zk = z_sb[:, kt * F:(kt + 1) * F]
                        if pair:
                            for nn in range(NB // MM_N):
                                h = nn % 2
                                nc.tensor.matmul(
                                    ps[h * F:(h + 1) * F,
                                       nn * MM_N:(nn + 1) * MM_N],
                                    lhsT=zk,
                                    rhs=at[:, t, nn * MM_N:(nn + 1) * MM_N],
                                    start=(kt == 0),
                                    stop=(kt == KT - 1),
                                    tile_position=(0, h * F),
                                )
                        else:
                            for nn in range(NB // MM_N):
                                nc.tensor.matmul(
                                    ps[:, nn * MM_N:(nn + 1) * MM_N],
                                    lhsT=zk,
                                    rhs=at[:, t, nn * MM_N:(nn + 1) * MM_N],
                                    start=(kt == 0),
                                    stop=(kt == KT - 1),
                                )

                out_sb = singles.tile([b_p, NB], mybir.dt.float32,
                                      tag="out_sb")
                relu = mybir.ActivationFunctionType.Relu
                if mode in ("dma", "dma8"):
                    nc.vector.tensor_copy(out_sb[:F, :F], z_sb[:F, :F])
                    nc.sync.dma_start(o_out.ap(), out_sb[:F, :])
                elif pair:
                    # touch only the written PSUM quadrants
                    for nn in range(NB // MM_N):
                        h = nn % 2
                        sp = slice(h * F, (h + 1) * F)
                        sf = slice(nn * MM_N, (nn + 1) * MM_N)
                        if fp8:
                            nc.vector.tensor_mul(out_sb[sp, sf], ps[sp, sf],
                                                 d_sb[sp, sf])
                            nc.scalar.activation(out_sb[sp, sf],
                                                 out_sb[sp, sf], relu,
                                                 bias=b_sb[sp], scale=1.0)
                        else:
                            nc.scalar.activation(out_sb[sp, sf], ps[sp, sf],
                                                 relu, bias=b_sb[sp],
                                                 scale=1.0)
                        nc.sync.dma_start(o_out.ap()[:, sf], out_sb[sp, sf])
                elif fp8:
                    nc.vector.tensor_mul(out_sb[:], ps[:], d_sb[:])
                    nc.scalar.activation(out_sb[:], out_sb[:], relu,
                                         bias=b_sb[:], scale=1.0)
                    nc.sync.dma_start(o_out.ap(), out_sb[:])
                else:
                    nc.scalar.activation(out_sb[:], ps[:], relu,
                                         bias=b_sb[:], scale=1.0)
                    nc.sync.dma_start(o_out.ap(), out_sb[:])

    nc.compile()
    _BASS_CACHE[reps] = nc
    return nc


def _host_prep(x, adj, W, b, mode=None):
    """Host-side sharding/preprocessing -> per-core input maps."""
    if mode is None:
        mode = MODE
    fp8 = mode in ("fp8", "fp8pair", "dma8", "mm8")
    pair = mode in ("pair", "fp8pair")
    x = np.asarray(x, dtype=np.float32)
    adj = np.asarray(adj, dtype=np.float32)
    W = np.asarray(W, dtype=np.float32)
    b = np.asarray(b, dtype=np.float32)

    deg = adj.sum(axis=0) + 1.0
    dinv = np.where(deg > 0, 1.0 / np.sqrt(deg), 0.0).astype(np.float32)

    z = (dinv[:, None] * (x @ W)).astype(np.float32)  # [N, F]
    # k-major layout: z_sb[p, kt*F + f] = z[kt*128 + p, f]
    z_dev = np.ascontiguousarray(
        z.reshape(KT, P, F).transpose(1, 0, 2).reshape(P, KT * F)
    ).astype(ml_dtypes.bfloat16)

    if pair:
        b_dev = np.ascontiguousarray(
            np.concatenate([b, b]).reshape(2 * F, 1))
    else:
        b_dev = np.ascontiguousarray(b.reshape(F, 1))

    def _pair_dinv(dc):
        # [128, NB]: chunk nn lives at [64*(nn%2):64*(nn%2+1), nn*512:...]
        d = np.zeros((2 * F, NB), np.float32)
        for nn in range(NB // MM_N):
            h = nn % 2
            d[h * F:(h + 1) * F, nn * MM_N:(nn + 1) * MM_N] = \
                dc[nn * MM_N:(nn + 1) * MM_N]
        return d

    in_maps = []
    idx = np.arange(NB)
    for c in range(NCORES):
        cs = c * NB
        if fp8:
            # adjacency stays exact {0,1,2} in fp8; dinv applied on device
            blk = adj[:, cs:cs + NB].copy()
            blk[cs + idx, idx] += 1.0  # self-loop (+I)
            dc = dinv[cs:cs + NB]
            m = {
                "a": blk.astype(ml_dtypes.float8_e4m3),
                "z": z_dev,
                "bvec": b_dev,
                "dinv": (_pair_dinv(dc) if pair else np.ascontiguousarray(
                    np.broadcast_to(dc, (F, NB)))),
            }
        else:
            blk = adj[:, cs:cs + NB] * dinv[cs:cs + NB][None, :]
            blk[cs + idx, idx] += dinv[cs + idx]  # fold self-loop (+I)
            m = {
                "a": blk.astype(ml_dtypes.bfloat16),
                "z": z_dev,
                "bvec": b_dev,
            }
        in_maps.append(m)
    return in_maps


def _assemble(results, mode=None):
    """Device outputs -> full [N, F] output."""
    if mode is None:
        mode = MODE
    out = np.empty((N, F), dtype=np.float32)
    for c in range(NCORES):
        out[c * NB:(c + 1) * NB, :] = results[c]["o"].T
    return out


def kernel(x, adj, W, b):
    from concourse import bass_utils

    nc = _build_bass(mode=MODE)
    in_maps = _host_prep(x, adj, W, b, mode=MODE)
    res = bass_utils.run_bass_kernel_spmd(nc, in_maps,
                                          core_ids=list(range(NCORES)))
    return _assemble(res.results, mode=MODE)
